# revision 39
# baseline (speedup 1.0000x reference)
"""GQA forward (b=2, s=2048, H=32 q heads, 8 kv heads, d=64) on 8 TRN2 cores.

Sharding: core k owns query heads 4k..4k+3 and kv head k. GQA group
structure makes attention fully local per core (q heads 4k..4k+3 attend
only to kv head k). x is replicated; W columns are sharded; outputs are
column-concatenated.

v6 layout (evolved from v5; all matmul operands bf16):
  - x transposed + bf16-cast on the HOST; x.T tiles DMA straight into SBUF.
  - Projections in TRANSPOSED layout; their matmuls are SOFTWARE-PIPELINED
    into the attention loops as PE filler: attention pair A's j-loop carries
    this tile's blk1 (q heads 2,3) projection matmuls, pair B's j-loop
    carries the NEXT tile's blk2 (K/V) and blk0 projection matmuls.  The PE
    queue therefore always has independent matmuls to run while the exp
    activation for the next attention step is still in flight, which also
    keeps the HAM clock-gate warm.
  - RoPE in transposed space via a host-side head-dim permutation that
    places each rotation partner 16 lanes away WITHIN a 32-partition
    quadrant: the cross-half term comes from a single DVE stream_shuffle;
    cos/sin tables are fp16 so all three RoPE tensor_tensor passes run in
    the DVE 2x all-2-byte mode.
  - Attention processes head pairs (h0,h1 | h2,h3): both heads of a pair
    share the moving operand qt[:,p,:]; kt_res = [K.T;0] selects the head
    using rows 0:64, kt2_res = [0;K.T] the one using rows 64:128.  One EXP
    activation covers both heads' strips ([128, 2, 512] PSUM tile).
  - Causal masking: diagonal blocks are exp'd UNMASKED (logits bounded, no
    overflow) and the forbidden triangle is zeroed AFTER exp by a DVE
    tensor_tensor multiply with a 0/1 bf16 mask (all-SBUF 2-byte -> 2x
    mode, and a much shorter cross-engine chain than GpSimd).
  - ctx.T[80,q] = [V|1|0pad].T @ P.T accumulated in PSUM per pair (row 64 =
    softmax sums); V transposed into the resident via DMA-transposes.
  - Finalize per pair: evict ctx.T to bf16, 8 PE-transposes into one
    single-bank PSUM tile [128,2,4,80], ONE batched reciprocal of the 8
    sums, ONE broadcast tensor_tensor multiply into the f32 output block,
    one DMA per s-tile.  Finalize is deferred by one pair so PE transposes
    never stall the next pair's matmuls.
"""

import numpy as np
from contextlib import ExitStack

import ml_dtypes

import concourse.bass as bass
import concourse.bacc as bacc
import concourse.mybir as mybir
from concourse import tile
from concourse.bass_utils import run_bass_kernel_spmd

F32 = mybir.dt.float32
F16 = mybir.dt.float16
BF16 = mybir.dt.bfloat16
MUL = mybir.AluOpType.mult
ADD = mybir.AluOpType.add
EXP = mybir.ActivationFunctionType.Exp

B = 2
S = 2048
DIN = 2048
D = 64              # head dim
HPC = 4             # query heads per core
NCORES = 8
WCOLS = 4 * D + D + D  # 256 q cols + 64 k + 64 v = 384
ST = 512            # s-tile (rows per outer step)
NST = B * S // ST   # 8 s-tiles
NCH = DIN // 128    # 16 k-chunks
NKV = S // 128      # kv tiles per batch

# intra-quadrant half swap: lanes 16:32 <-> 0:16 of every 32-partition group
SWAP16 = list(range(16, 32)) + list(range(16))


def build_bass():
    nc = bacc.Bacc(None, target_bir_lowering=False)
    # x.T pre-tiled on the host as [tile, partition, chunk, s] so every
    # per-partition DMA run is 16KB contiguous (a flat [DIN, B*S] layout
    # gives 1KB strided runs that cap transfers at ~65 GB/s)
    xt_d = nc.declare_dram_parameter("xt", [NST * 128, NCH * ST], BF16,
                                     isOutput=False)
    w_d = nc.declare_dram_parameter("w", [DIN, WCOLS], BF16, isOutput=False)
    cos_d = nc.declare_dram_parameter("cost", [128, S], F16, isOutput=False)
    sin_d = nc.declare_dram_parameter("sint", [128, S], F16, isOutput=False)
    tri_d = nc.declare_dram_parameter("tri", [128, 128], BF16, isOutput=False)
    id_d = nc.declare_dram_parameter("ident", [128, 128], BF16, isOutput=False)
    sh_d = nc.declare_dram_parameter("shft", [128, 128], BF16, isOutput=False)
    out_d = nc.declare_dram_parameter("out", [B * S, HPC * D], F32, isOutput=True)

    with ExitStack() as ctx:
        tc = ctx.enter_context(tile.TileContext(nc))
        const = ctx.enter_context(tc.tile_pool(name="const", bufs=1))
        resid = ctx.enter_context(tc.tile_pool(name="resid", bufs=1))
        xt_p = ctx.enter_context(tc.tile_pool(name="xt", bufs=3))
        qn_p = ctx.enter_context(tc.tile_pool(name="qn", bufs=3))
        qt_p = ctx.enter_context(tc.tile_pool(name="qt", bufs=2))
        p_p = ctx.enter_context(tc.tile_pool(name="p", bufs=3))
        cxs_p = ctx.enter_context(tc.tile_pool(name="cxs", bufs=2))
        fo_p = ctx.enter_context(tc.tile_pool(name="fo", bufs=2))
        ob_p = ctx.enter_context(tc.tile_pool(name="ob", bufs=2))
        pr_ps = ctx.enter_context(tc.tile_pool(name="pr_ps", bufs=1, space="PSUM"))
        sc_ps = ctx.enter_context(tc.tile_pool(name="sc_ps", bufs=2, space="PSUM"))
        cx_ps = ctx.enter_context(tc.tile_pool(name="cx_ps", bufs=1, space="PSUM"))
        fi_ps = ctx.enter_context(tc.tile_pool(name="fi_ps", bufs=1, space="PSUM"))

        # constants / residents.  The weight chunks and the first x.T tile
        # gate the first matmuls; both are split across all four DMA-capable
        # queues so the transfers run on parallel DMA engines instead of
        # serializing behind each other on the Sync queue.
        w_view = w_d.rearrange("(c p) n -> p c n", p=128)
        xt_view = xt_d.rearrange("(t p) (c s) -> t p c s", p=128, s=ST)
        w_sb = const.tile([128, NCH, WCOLS], BF16)
        xt0 = xt_p.tile([128, NCH, ST], BF16, name="xt")
        # startup: the first tile's projections are DMA-bound; the weight
        # chunks and first x.T tile split across the three DMA-capable
        # queues so the transfers run on parallel DMA engines.
        nc.sync.dma_start(out=w_sb[:, 0:2, :], in_=w_view[:, 0:2, :])
        nc.sync.dma_start(out=xt0[:, 0:4, :], in_=xt_view[0, :, 0:4, :])
        nc.gpsimd.dma_start(out=xt0[:, 4:10, :], in_=xt_view[0, :, 4:10, :])
        nc.scalar.dma_start(out=xt0[:, 10:16, :],
                            in_=xt_view[0, :, 10:16, :])
        nc.sync.dma_start(out=w_sb[:, 2:NCH, :], in_=w_view[:, 2:NCH, :])
        tri_sb = const.tile([128, 128], BF16)
        nc.gpsimd.dma_start(out=tri_sb[:], in_=tri_d[:])
        shft = const.tile([128, 128], BF16)
        nc.gpsimd.dma_start(out=shft[:], in_=sh_d[:])
        ident = const.tile([128, 128], BF16)
        nc.gpsimd.dma_start(out=ident[:], in_=id_d[:])
        cos_sb = const.tile([128, S], F16)
        nc.gpsimd.dma_start(out=cos_sb[:], in_=cos_d[:])
        sin_sb = const.tile([128, S], F16)
        nc.gpsimd.dma_start(out=sin_sb[:], in_=sin_d[:])

        # Two K.T residents, zero-padded to k=128 so the scores matmul always
        # contracts over the full partition range with the full Q pair tile.
        kt_res = resid.tile([128, B * S], BF16)
        kt2_res = resid.tile([128, B * S], BF16)
        nc.vector.memset(kt_res[64:128, :], 0.0)
        # kt2_res needs no memset: the per-tile shift-matmul eviction writes
        # all 128 rows of its column range (zeros land in rows 0:64)
        # [V | 1 | 0-pad] per kv tile: col 64 = ones (softmax sums land in
        # ctx.T row 64), cols 65:80 zero so ctx.T rows 65:80 read initialized
        vp_res = resid.tile([128, B * NKV, 80], BF16)
        nc.vector.memset(vp_res[:, :, 64:65], 1.0)
        nc.vector.memset(vp_res[:, :, 65:80], 0.0)

        def load_xt(st):
            xt = xt_p.tile([128, NCH, ST], BF16, name="xt")
            for cc in range(0, NCH, 8):
                nc.sync.dma_start(out=xt[:, cc:cc + 8, :],
                                  in_=xt_view[st, :, cc:cc + 8, :])
            return xt

        def proj_mms(blk, xt):
            """Generator: yields after each of the 16 projection matmuls."""
            pp = pr_ps.tile([128, ST], F32, tag="pp", name="pp")
            for c in range(NCH):
                nc.tensor.matmul(
                    pp[:], w_sb[:, c, blk * 128:(blk + 1) * 128],
                    xt[:, c, :], start=(c == 0), stop=(c == NCH - 1))
                yield pp
            while True:
                yield pp

        def proj_tail(blk, pp, qt, st):
            b, sti = divmod(st, 4)
            ssl = slice(st * ST, (st + 1) * ST)
            tsl = slice(sti * ST, (sti + 1) * ST)
            raw = qn_p.tile([128, ST], BF16, tag="raw", name="raw")
            nc.vector.tensor_copy(raw[:], pp[:])
            swp = qn_p.tile([128, ST], BF16, tag="swp", name="swp")
            nc.vector.stream_shuffle(swp[:], raw[:], SWAP16)
            ts = qn_p.tile([128, ST], BF16, tag="ts", name="ts")
            if blk < 2:
                dst = qt[:, blk, :]
                nc.vector.tensor_tensor(dst, raw[:], cos_sb[:, tsl], MUL)
                nc.vector.tensor_tensor(ts[:], swp[:], sin_sb[:, tsl], MUL)
                nc.vector.tensor_tensor(dst, dst, ts[:], ADD)
            else:
                kd = kt_res[0:64, ssl]
                nc.vector.tensor_tensor(kd, raw[0:64, :],
                                        cos_sb[0:64, tsl], MUL)
                nc.vector.tensor_tensor(ts[0:64, :], swp[0:64, :],
                                        sin_sb[0:64, tsl], MUL)
                nc.vector.tensor_tensor(kd, kd, ts[0:64, :], ADD)
                # V.T rows -> natural [kv, d] into the resident
                for ptb in range(4):
                    nc.sync.dma_start(
                        out=vp_res[:, b * NKV + sti * 4 + ptb, 0:64],
                        in_=raw[64:128, ptb * 128:(ptb + 1) * 128],
                        transpose=True)
        # kt2_res = [0; K.T] via a 64-row shift matmul through PSUM (a
        # SBUF->SBUF DMA here stalls attention behind big x.T / output
        # transfers on the DMA engines).  Emitted separately, well after the
        # K-RoPE chain, so the PE never waits on it.
        def emit_kt2_shift(st):
            ssl = slice(st * ST, (st + 1) * ST)
            ppk = sc_ps.tile([128, ST], F32, tag="sc", name="ppk")
            nc.tensor.matmul(ppk[:], shft[:], kt_res[:, ssl],
                             start=True, stop=True)
            nc.vector.tensor_copy(kt2_res[:, ssl], ppk[:])

        # finalize is split: the PSUM eviction CAST is emitted at the owning
        # pair's end, while the PE transposes + normalization are deferred by
        # one pair -- so the CAST has a whole pair of attention to complete
        # and the transposes never stall the PE queue waiting on it.
        def emit_finalize(cxs, p, ob, st_):
            fi = fi_ps.tile([128, 2, 4, 80], BF16, tag="fi", name="fi")
            for pi in range(2):
                for qq in range(4):
                    nc.tensor.transpose(fi[:, pi, qq, :],
                                        cxs[:, pi, qq * 128:(qq + 1) * 128],
                                        ident[0:80, 0:80])
            rv = fo_p.tile([128, 2, 4, 1], F32, tag="rv", name="rv")
            nc.vector.reciprocal(rv[:], fi[:, :, :, 64:65])
            # one multiply for both heads of the pair: free dims (qq, pi, d)
            nc.vector.tensor_tensor(
                ob[:, :, 2 * p * D:(2 * p + 2) * D].rearrange(
                    "t q (h c) -> t q h c", h=2),
                fi[:, :, :, 0:D].transpose([0, 2, 1, 3]),
                rv[:, :, :, :].transpose([0, 2, 1, 3]).broadcast_to(
                    [128, 4, 2, D]),
                MUL)
            if p == 1:
                nc.sync.dma_start(
                    out=out_d[st_ * ST:(st_ + 1) * ST, :].rearrange(
                        "(q p) n -> p q n", p=128),
                    in_=ob[:])

        pending = None
        tri2 = tri_sb[:].unsqueeze(1).broadcast_to([128, 2, 128])

        def attention(p, qt, ob, st, fillers):
            """fillers: list of (weight, step_fn) pseudo-generators; weight
            gives how many steps to pull per attention j-step."""
            nonlocal pending
            b, sti = divmod(st, 4)
            nblk = 4 * sti + 4

            def kt1(j):
                return kt_res[:, b * S + j * 128:b * S + (j + 1) * 128]

            def kt2(j):
                return kt2_res[:, b * S + j * 128:b * S + (j + 1) * 128]

            def vp(j):
                return vp_res[:, b * NKV + j, :]

            cxt = cx_ps.tile([128, 2, ST], F32, tag="cxt", name="cxt")
            for j in range(nblk):
                r = j - 4 * sti                        # >=0 -> diagonal block
                q0 = 0 if r < 0 else 128 * r
                sc = sc_ps.tile([128, 2, ST], F32, tag="sc", name="sc")
                nc.tensor.matmul(sc[:, 0, q0:ST], kt1(j), qt[:, p, q0:ST],
                                 start=True, stop=True)
                nc.tensor.matmul(sc[:, 1, q0:ST], kt2(j), qt[:, p, q0:ST],
                                 start=True, stop=True)
                psb = p_p.tile([128, 2, ST], BF16, tag="psb", name="psb")
                nc.scalar.activation(psb[:, :, q0:ST], sc[:, :, q0:ST],
                                     EXP, scale=0.125)
                if r >= 0:
                    msk = psb[:, :, q0:q0 + 128]
                    nc.vector.tensor_tensor(msk, msk, tri2, MUL)
                for pi in range(2):
                    nc.tensor.matmul(
                        cxt[0:80, pi, q0:ST], vp(j), psb[:, pi, q0:ST],
                        start=(j == 0), stop=(j == nblk - 1))
                for weight, step in fillers:
                    for _ in range(weight):
                        next(step, None)

            for weight, step in fillers:
                while next(step, None) is not None:
                    pass

            # evict this pair's ctx.T now; the rest of its finalize runs at
            # the NEXT pair's end (one full pair of CAST lead time)
            cxs = cxs_p.tile([80, 2, ST], BF16, name="cxs")
            nc.vector.tensor_copy(cxs[:], cxt[0:80, :, :])
            if pending is not None:
                emit_finalize(*pending)
            pending = (cxs, p, ob, st)

        def fin_gen(gen, n):
            """Wrap a proj_mms generator to yield its 16 MMs then stop."""
            for _ in range(n):
                yield next(gen)

        # ---- prologue: tile 0's K/V and blk0 projections, unoverlapped ----
        xts = {0: xt0}
        qts = {0: qt_p.tile([128, 2, ST], BF16, tag="qt", name="qt")}
        g = proj_mms(2, xts[0])
        pp = None
        for _ in range(NCH):
            pp = next(g)
        proj_tail(2, pp, None, 0)
        emit_kt2_shift(0)
        g = proj_mms(0, xts[0])
        for _ in range(NCH):
            pp = next(g)
        proj_tail(0, pp, qts[0], 0)

        for st in range(NST):
            b, sti = divmod(st, 4)
            nblk = 4 * sti + 4
            ob = ob_p.tile([128, 4, HPC * D], F32, name="ob")

            # pair A, interleaved with this tile's blk1 projection
            g1 = proj_mms(1, xts[st])
            w1 = max(1, -(-NCH // nblk))
            steps1 = fin_gen(g1, NCH)
            attention(0, qts[st], ob, st, [(w1, steps1)])
            proj_tail(1, next(g1), qts[st], st)

            # pair B, interleaved with next tile's blk2 + blk0 projections
            fillers = []
            if st + 1 < NST:
                xts[st + 1] = load_xt(st + 1)
                qts[st + 1] = qt_p.tile([128, 2, ST], BF16, tag="qt",
                                        name="qt")
                g2 = proj_mms(2, xts[st + 1])
                g0 = proj_mms(0, xts[st + 1])

                def chain(g2=g2, g0=g0, nst=st + 1):
                    for _ in range(NCH):
                        yield next(g2)
                    proj_tail(2, next(g2), None, nst)
                    # let the blk2 eviction CAST complete before the first
                    # blk0 matmul (same single-bank PSUM slot) hits the PE
                    for _ in range(4):
                        yield "pause"
                    for _ in range(NCH):
                        yield next(g0)

                wc = max(1, -(-(2 * NCH) // nblk))
                ch = chain()
                fillers = [(wc, ch)]
            attention(1, qts[st], ob, st, fillers)
            if st + 1 < NST:
                proj_tail(0, next(g0), qts[st + 1], st + 1)
                emit_kt2_shift(st + 1)

        emit_finalize(*pending)
    return nc


_NC_CACHE = None

# Head-dim permutation shared by Q and K (scores are invariant to it).
# Row r of a 64-row head block holds original coordinate PERM64[r]:
# quadrant q = r//32, half h = (r%32)//16, lane l = r%16 ->
# rotation-pair index p = q*16+l, even/odd = h.  Each RoPE partner is then
# 16 lanes away inside the same 32-partition quadrant, which is exactly
# what a single DVE stream_shuffle can swap.
_RR = np.arange(D)
_PERM64 = 2 * ((_RR // 32) * 16 + (_RR % 16)) + (_RR % 32) // 16


def _host_consts():
    freqs = 1.0 / (10000.0 ** (np.arange(32, dtype=np.float64) * 2 / D))
    ang = freqs[:, None] * np.arange(S, dtype=np.float64)[None, :]  # (32, S)
    cos32 = np.cos(ang)
    sin32 = np.sin(ang)
    rr = np.arange(128) % D
    pair = (rr // 32 % 2) * 16 + rr % 16
    half = (rr % 32) // 16
    cosT = cos32[pair, :].astype(np.float16)                   # (128, S)
    sgn = np.where(half == 0, -1.0, 1.0)[:, None]
    sinT = (sin32[pair, :] * sgn).astype(np.float16)           # (128, S)
    kv, qq = np.meshgrid(np.arange(128), np.arange(128), indexing="ij")
    tri01 = (kv <= qq).astype(np.float32).astype(ml_dtypes.bfloat16)
    ident = np.eye(128, dtype=np.float32).astype(ml_dtypes.bfloat16)
    # shift-by-64: (shft.T @ x)[m] = x[m-64] for m>=64, else 0
    shft = np.zeros((128, 128), dtype=np.float32)
    shft[np.arange(64), np.arange(64) + 64] = 1.0
    shft = shft.astype(ml_dtypes.bfloat16)
    return cosT, sinT, tri01, ident, shft


def _in_maps(x, Wq, Wk, Wv):
    x = np.asarray(x, dtype=np.float32).reshape(B * S, DIN)
    # pre-tiled transposed layout [tile, partition, chunk, s] -> contiguous
    # 16KB per-partition DMA runs (see xt_d declaration)
    xt = np.ascontiguousarray(
        x.reshape(NST, ST, NCH, 128).transpose(0, 3, 2, 1)
    ).astype(ml_dtypes.bfloat16).reshape(NST * 128, NCH * ST)
    Wq = np.asarray(Wq, dtype=np.float32)
    Wk = np.asarray(Wk, dtype=np.float32)
    Wv = np.asarray(Wv, dtype=np.float32)
    # permute head-dims of Q and K weights for quadrant-local rotate-half
    Wq = Wq.reshape(DIN, 32, D)[:, :, _PERM64].reshape(DIN, 32 * D)
    Wk = Wk.reshape(DIN, 8, D)[:, :, _PERM64].reshape(DIN, 8 * D)
    cosT, sinT, tri01, ident, shft = _host_consts()

    in_maps = []
    for k in range(NCORES):
        w_all = np.hstack([
            Wq[:, k * 256:(k + 1) * 256],
            Wk[:, k * 64:(k + 1) * 64],
            Wv[:, k * 64:(k + 1) * 64],
        ]).astype(ml_dtypes.bfloat16)
        in_maps.append({
            "xt": xt, "w": np.ascontiguousarray(w_all),
            "cost": cosT, "sint": sinT, "tri": tri01, "ident": ident,
            "shft": shft,
        })
    return in_maps


def _run(in_maps, **kwargs):
    global _NC_CACHE
    if _NC_CACHE is None:
        _NC_CACHE = build_bass()
        _NC_CACHE.finalize()
    return run_bass_kernel_spmd(_NC_CACHE, in_maps, list(range(NCORES)),
                                **kwargs)


def kernel(x, Wq, Wk, Wv):
    res = _run(_in_maps(x, Wq, Wk, Wv))
    out = np.concatenate([res.results[k]["out"] for k in range(NCORES)], axis=1)
    return out.reshape(B, S, 32 * D)
